# revision 3
# baseline (speedup 1.0000x reference)
"""CAM (channel attention) module kernel for Trainium2, 8 NeuronCores.

Reference computation (per batch b):
    q = x[b].reshape(C, N)                      # C=128, N=65536
    energy = q @ q.T                            # C x C
    att = softmax(rowmax(energy) - energy)      # == exp(rowmin(e)-e)/rowsum
    out = att @ q
    result = gamma * out + x

Sharding: cores 0-3 handle batch 0, cores 4-7 handle batch 1; each core
owns a contiguous N/4 = 16384 column slice.  Partial C x C energy is
AllReduced within each group of 4, softmax is computed redundantly
(tiny), and the AV matmul + residual are done on the local slice.

Numerics: the PE matmuls run fp16 with an hi/lo split for the energy
term:  q = qh + ql (fp16 each, ~22 mantissa bits combined), and
    E = Qh Qh^T + C + C^T,   C = sum_j Qh_j Ql_j^T
which keeps the absolute error of the 65536-length dot products small
enough for the softmax (exp) stage.  The residual add uses the exact
f32 copy of x.  gamma is folded into the attention matrix
(att_scaled = gamma*att), so the residual is a single add.
"""

import numpy as np

import concourse.bass as bass
import concourse.mybir as mybir
import concourse.tile as tile
from concourse import bacc
from concourse.bass_utils import run_bass_kernel_spmd
from concourse.masks import make_identity

B, C, D, H, W = 2, 128, 16, 64, 64
N = D * H * W  # 65536
NCORES = 8
SHARDS_PER_BATCH = 4
NS = N // SHARDS_PER_BATCH  # 16384 columns per core
NB = 2048                   # n-block size for the load/cast/transpose pipeline
JCH = NS // 128             # 128 transposed chunks per core
AVF = 512                   # AV matmul free-dim chunk

F32 = mybir.dt.float32
F16 = mybir.dt.float16


def _body(nc: bass.Bass, tc: "tile.TileContext", xs, gm, out):
    with (
        tc.tile_pool(name="big", bufs=1) as big,
        tc.tile_pool(name="small", bufs=1) as small,
        tc.tile_pool(name="work", bufs=4) as work,
        tc.tile_pool(name="qlb", bufs=3) as qlb,
        tc.tile_pool(name="psum_e", bufs=1, space="PSUM") as pse,
        tc.tile_pool(name="psum_av", bufs=4, space="PSUM") as psav,
        tc.tile_pool(name="dram", bufs=1, space="DRAM") as dram,
    ):
        # Persistent SBUF tensors
        xf = big.tile([C, NS], F32, tag="xf")        # exact f32 x (residual)
        qh = big.tile([C, NS], F16, tag="qh")        # fp16 hi, natural (AV rhs)
        qhT = big.tile([128, JCH, 128], F16, tag="qhT")  # n-partition chunks
        qlT = big.tile([128, JCH, 128], F16, tag="qlT")

        ident = small.tile([128, 128], F32, tag="ident")
        make_identity(nc, ident)

        g0 = small.tile([1, 1], F32, tag="g0")
        gsb = small.tile([128, 1], F32, tag="gsb")
        nc.sync.dma_start(g0[:], gm[None, :])
        nc.gpsimd.partition_broadcast(gsb, g0[:])

        # ---- Stage 1: load -> split-cast -> xbar-transpose -> energy MMs ----
        e_ps = pse.tile([128, 128], F32, tag="e_ps")   # Qh Qh^T accumulator
        c_ps = pse.tile([128, 128], F32, tag="c_ps")   # Qh Ql^T accumulator
        nblk = NS // NB
        jpb = NB // 128
        for blk in range(nblk):
            sl = slice(blk * NB, (blk + 1) * NB)
            jsl = slice(blk * jpb, (blk + 1) * jpb)
            nc.sync.dma_start(xf[:, sl], xs[:, sl])
            nc.vector.tensor_copy(qh[:, sl], xf[:, sl])          # fp16 hi
            ql = qlb.tile([C, NB], F16, tag="ql")
            nc.vector.tensor_tensor(                              # fp16 lo
                ql, xf[:, sl], qh[:, sl], mybir.AluOpType.subtract
            )
            nc.sync.dma_start(qhT[:, jsl, :], qh[:, sl], transpose=True)
            nc.sync.dma_start(qlT[:, jsl, :], ql, transpose=True)
            for j in range(blk * jpb, (blk + 1) * jpb):
                first = j == 0
                last = j == JCH - 1
                nc.tensor.matmul(
                    e_ps, lhsT=qhT[:, j, :], rhs=qhT[:, j, :],
                    start=first, stop=last,
                )
                nc.tensor.matmul(
                    c_ps, lhsT=qhT[:, j, :], rhs=qlT[:, j, :],
                    start=first, stop=last,
                )

        # ---- Stage 2: E = e_ps + c_ps + c_ps^T, then AllReduce ----
        c_sb = small.tile([128, 128], F32, tag="c_sb")
        nc.vector.tensor_copy(c_sb, c_ps)
        cT_ps = pse.tile([128, 128], F32, tag="cT_ps")
        nc.tensor.transpose(cT_ps, c_sb, ident)
        e_sb = small.tile([128, 128], F32, tag="e_sb")
        nc.vector.tensor_add(e_sb, e_ps, c_sb)
        nc.vector.tensor_add(e_sb, e_sb, cT_ps)

        e_in = dram.tile([128, 128], F32, tag="e_in")
        e_out = dram.tile([128, 128], F32, tag="e_out")
        nc.sync.dma_start(e_in[:], e_sb)
        nc.gpsimd.collective_compute(
            "AllReduce",
            mybir.AluOpType.add,
            replica_groups=[[0, 1, 2, 3], [4, 5, 6, 7]],
            ins=[e_in.opt()],
            outs=[e_out.opt()],
        )
        e_full = small.tile([128, 128], F32, tag="e_full")
        nc.sync.dma_start(e_full, e_out[:])

        # ---- Stage 3: softmax (att = exp(rowmin(e) - e) / rowsum) ----
        m = small.tile([128, 1], F32, tag="m")
        nc.vector.tensor_reduce(
            m, e_full, axis=mybir.AxisListType.X, op=mybir.AluOpType.min
        )
        t = small.tile([128, 128], F32, tag="t")
        r = small.tile([128, 1], F32, tag="r")
        nc.scalar.activation(
            t, e_full, mybir.ActivationFunctionType.Exp,
            bias=m, scale=-1.0, accum_out=r,
        )
        rinv = small.tile([128, 1], F32, tag="rinv")
        nc.vector.reciprocal(rinv, r)
        gr = small.tile([128, 1], F32, tag="gr")
        nc.vector.tensor_mul(gr, rinv, gsb)
        att = small.tile([128, 128], F32, tag="att")
        nc.vector.tensor_scalar_mul(att, t, gr)   # att = gamma * softmax rows

        attT_ps = pse.tile([128, 128], F32, tag="attT_ps")
        nc.tensor.transpose(attT_ps, att, ident)
        attT = small.tile([128, 128], F16, tag="attT")
        nc.vector.tensor_copy(attT, attT_ps)

        # ---- Stage 4: AV matmul + residual + store ----
        for f in range(NS // AVF):
            sl = slice(f * AVF, (f + 1) * AVF)
            av_ps = psav.tile([128, AVF], F32, tag="av_ps")
            nc.tensor.matmul(av_ps, lhsT=attT, rhs=qh[:, sl], start=True, stop=True)
            o_sb = work.tile([128, AVF], F32, tag="o_sb")
            nc.vector.tensor_add(o_sb, av_ps, xf[:, sl])
            nc.sync.dma_start(out[:, sl], o_sb)


_cached_nc = None


def _build():
    nc = bacc.Bacc(
        "TRN2",
        target_bir_lowering=False,
        debug=False,
        enable_asserts=False,
        num_devices=NCORES,
    )
    xs = nc.dram_tensor("xs", [C, NS], F32, kind="ExternalInput").ap()
    gm = nc.dram_tensor("gamma", [1], F32, kind="ExternalInput").ap()
    out = nc.dram_tensor("out", [C, NS], F32, kind="ExternalOutput").ap()
    with tile.TileContext(nc) as tc:
        _body(nc, tc, xs, gm, out)
    nc.compile()
    return nc


def kernel(x: np.ndarray, gamma: np.ndarray, _collect_results=None) -> np.ndarray:
    global _cached_nc
    if _cached_nc is None:
        _cached_nc = _build()
    nc = _cached_nc

    xr = np.ascontiguousarray(np.asarray(x, dtype=np.float32).reshape(B, C, N))
    gamma = np.ascontiguousarray(np.asarray(gamma, dtype=np.float32))
    in_maps = []
    for k in range(NCORES):
        b, s = divmod(k, SHARDS_PER_BATCH)
        shard = np.ascontiguousarray(xr[b, :, s * NS:(s + 1) * NS])
        in_maps.append({"xs": shard, "gamma": gamma})

    res = run_bass_kernel_spmd(nc, in_maps, core_ids=list(range(NCORES)))
    if _collect_results is not None:
        _collect_results.append(res)

    outf = np.empty((B, C, N), np.float32)
    for k in range(NCORES):
        b, s = divmod(k, SHARDS_PER_BATCH)
        outf[b, :, s * NS:(s + 1) * NS] = res.results[k]["out"]
    return outf.reshape(B, C, D, H, W)


# revision 10
# speedup vs baseline: 1.0388x; 1.0388x over previous
"""CAM (channel attention) module kernel for Trainium2, 8 NeuronCores.

Reference computation (per batch b):
    q = x[b].reshape(C, N)                      # C=128, N=65536
    energy = q @ q.T                            # C x C
    att = softmax(rowmax(energy) - energy)      # == exp(rowmin(e)-e)/rowsum
    out = att @ q
    result = gamma * out + x

Sharding: cores 0-3 handle batch 0, cores 4-7 handle batch 1; each core
owns a contiguous N/4 = 16384 column slice.  Partial C x C energy is
AllReduced within each group of 4, softmax is computed redundantly
(tiny), and the AV matmul + residual are done on the local slice.

Numerics: the PE matmuls run fp16 with an hi/lo split for the energy
term:  q = qh + ql (fp16 each, ~22 mantissa bits combined), and
    E = Qh Qh^T + C + C^T,   C = sum_j Qh_j Ql_j^T
which keeps the absolute error of the 65536-length dot products small
enough for the softmax (exp) stage.  The residual add uses the exact
f32 copy of x.  gamma is folded into the attention matrix
(att_scaled = gamma*att), so the residual is a single add.
"""

import numpy as np

import concourse.bass as bass
import concourse.mybir as mybir
import concourse.tile as tile
from concourse import bacc
from concourse.bass_utils import run_bass_kernel_spmd
from concourse.masks import make_identity

B, C, D, H, W = 2, 128, 16, 64, 64
N = D * H * W  # 65536
NCORES = 8
SHARDS_PER_BATCH = 4
NS = N // SHARDS_PER_BATCH  # 16384 columns per core
NB = 1024                   # n-block size for the load/cast/transpose pipeline
JCH = NS // 128             # 128 transposed chunks per core
AVF = 512                   # AV matmul free-dim chunk

F32 = mybir.dt.float32
F16 = mybir.dt.float16


def _body(nc: bass.Bass, tc: "tile.TileContext", xs, gm, out):
    with (
        tc.tile_pool(name="big", bufs=1) as big,
        tc.tile_pool(name="small", bufs=1) as small,
        tc.tile_pool(name="work", bufs=4) as work,
        tc.tile_pool(name="qlb", bufs=3) as qlb,
        tc.tile_pool(name="psum_e", bufs=1, space="PSUM") as pse,
        tc.tile_pool(name="psum_av", bufs=5, space="PSUM") as psav,
        tc.tile_pool(name="dram", bufs=1, space="DRAM") as dram,
    ):
        # Persistent SBUF tensors
        xf = big.tile([C, NS], F32, tag="xf")        # exact f32 x (residual)
        qh = big.tile([C, NS], F16, tag="qh")        # fp16 hi, natural (AV rhs)
        qhT = big.tile([128, JCH, 128], F16, tag="qhT")  # n-partition chunks
        qlT = big.tile([128, JCH, 128], F16, tag="qlT")

        ident = small.tile([128, 128], F32, tag="ident")
        make_identity(nc, ident)

        g0 = small.tile([1, 1], F32, tag="g0")
        gsb = small.tile([128, 1], F32, tag="gsb")
        nc.sync.dma_start(g0[:], gm[None, :])
        nc.gpsimd.partition_broadcast(gsb, g0[:])

        # ---- Stage 1: load -> split-cast -> xbar-transpose -> energy MMs ----
        e_ps = pse.tile([128, 128], F32, tag="e_ps")   # Qh Qh^T accumulator
        c_ps = pse.tile([128, 128], F32, tag="c_ps")   # Qh Ql^T accumulator
        nblk = NS // NB
        jpb = NB // 128
        for blk in range(nblk):
            sl = slice(blk * NB, (blk + 1) * NB)
            jsl = slice(blk * jpb, (blk + 1) * jpb)
            # input loads on SWDGE (gpsimd) so the two HWDGE queues
            # (sync/scalar) are dedicated to the xbar transposes
            nc.gpsimd.dma_start(xf[:, sl], xs[:, sl])
            nc.vector.tensor_copy(qh[:, sl], xf[:, sl])          # fp16 hi
            ql = qlb.tile([C, NB], F16, tag="ql")
            nc.vector.tensor_tensor(                              # fp16 lo
                ql, xf[:, sl], qh[:, sl], mybir.AluOpType.subtract
            )
            nc.sync.dma_start(qhT[:, jsl, :], qh[:, sl], transpose=True)
            nc.scalar.dma_start(qlT[:, jsl, :], ql, transpose=True)
            for j in range(blk * jpb, (blk + 1) * jpb):
                first = j == 0
                last = j == JCH - 1
                nc.tensor.matmul(
                    e_ps, lhsT=qhT[:, j, :], rhs=qhT[:, j, :],
                    start=first, stop=last,
                )
                nc.tensor.matmul(
                    c_ps, lhsT=qhT[:, j, :], rhs=qlT[:, j, :],
                    start=first, stop=last,
                )

        # ---- Stage 2: E = e_ps + c_ps + c_ps^T, then AllReduce ----
        c_sb = small.tile([128, 128], F32, tag="c_sb")
        nc.vector.tensor_copy(c_sb, c_ps)
        cT_ps = pse.tile([128, 128], F32, tag="tr_ps")
        nc.tensor.transpose(cT_ps, c_sb, ident)
        e_sb = small.tile([128, 128], F32, tag="e_sb")
        nc.vector.tensor_add(e_sb, e_ps, c_sb)
        nc.vector.tensor_add(e_sb, e_sb, cT_ps)

        e_in = dram.tile([128, 128], F32, tag="e_in")
        e_out = dram.tile([128, 128], F32, tag="e_out")
        nc.sync.dma_start(e_in[:], e_sb)
        nc.gpsimd.collective_compute(
            "AllReduce",
            mybir.AluOpType.add,
            replica_groups=[[0, 1, 2, 3], [4, 5, 6, 7]],
            ins=[e_in.opt()],
            outs=[e_out.opt()],
        )
        e_full = small.tile([128, 128], F32, tag="e_full")
        nc.sync.dma_start(e_full, e_out[:])

        # ---- Stage 3: softmax (att = exp(rowmin(e) - e) / rowsum) ----
        m = small.tile([128, 1], F32, tag="m")
        nc.vector.tensor_reduce(
            m, e_full, axis=mybir.AxisListType.X, op=mybir.AluOpType.min
        )
        t = small.tile([128, 128], F32, tag="t")
        r = small.tile([128, 1], F32, tag="r")
        nc.scalar.activation(
            t, e_full, mybir.ActivationFunctionType.Exp,
            bias=m, scale=-1.0, accum_out=r,
        )
        rinv = small.tile([128, 1], F32, tag="rinv")
        nc.vector.reciprocal(rinv, r)
        gr = small.tile([128, 1], F32, tag="gr")
        nc.vector.tensor_mul(gr, rinv, gsb)
        att = small.tile([128, 128], F32, tag="att")
        nc.vector.tensor_scalar_mul(att, t, gr)   # att = gamma * softmax rows

        attT_ps = pse.tile([128, 128], F32, tag="tr_ps")
        nc.tensor.transpose(attT_ps, att, ident)
        attT = small.tile([128, 128], F16, tag="attT")
        nc.vector.tensor_copy(attT, attT_ps)

        # ---- Stage 4: AV matmul + residual + store ----
        # residual adds alternate DVE / GpSimd to split the elementwise
        # load; stores alternate the two HWDGE queues
        for f in range(NS // AVF):
            sl = slice(f * AVF, (f + 1) * AVF)
            av_ps = psav.tile([128, AVF], F32, tag="av_ps")
            nc.tensor.matmul(av_ps, lhsT=attT, rhs=qh[:, sl], start=True, stop=True)
            o_sb = work.tile([128, AVF], F32, tag="o_sb")
            nc.vector.tensor_add(o_sb, av_ps, xf[:, sl])
            dma_eng = nc.sync if f % 2 else nc.scalar
            dma_eng.dma_start(out[:, sl], o_sb)


_cached_nc = None


def _build():
    nc = bacc.Bacc(
        "TRN2",
        target_bir_lowering=False,
        debug=False,
        enable_asserts=False,
        num_devices=NCORES,
    )
    xs = nc.dram_tensor("xs", [C, NS], F32, kind="ExternalInput").ap()
    gm = nc.dram_tensor("gamma", [1], F32, kind="ExternalInput").ap()
    out = nc.dram_tensor("out", [C, NS], F32, kind="ExternalOutput").ap()
    with tile.TileContext(nc) as tc:
        _body(nc, tc, xs, gm, out)
    nc.compile()
    return nc


def kernel(x: np.ndarray, gamma: np.ndarray, _collect_results=None) -> np.ndarray:
    global _cached_nc
    if _cached_nc is None:
        _cached_nc = _build()
    nc = _cached_nc

    xr = np.ascontiguousarray(np.asarray(x, dtype=np.float32).reshape(B, C, N))
    gamma = np.ascontiguousarray(np.asarray(gamma, dtype=np.float32))
    in_maps = []
    for k in range(NCORES):
        b, s = divmod(k, SHARDS_PER_BATCH)
        shard = np.ascontiguousarray(xr[b, :, s * NS:(s + 1) * NS])
        in_maps.append({"xs": shard, "gamma": gamma})

    res = run_bass_kernel_spmd(nc, in_maps, core_ids=list(range(NCORES)))
    if _collect_results is not None:
        _collect_results.append(res)

    outf = np.empty((B, C, N), np.float32)
    for k in range(NCORES):
        b, s = divmod(k, SHARDS_PER_BATCH)
        outf[b, :, s * NS:(s + 1) * NS] = res.results[k]["out"]
    return outf.reshape(B, C, D, H, W)
